# revision 4
# baseline (speedup 1.0000x reference)
"""Multi-head attention TRN2 kernel (b=4, n=2048, e=768, h=8 heads, d=96).

Sharding: 8 cores = 4 batches x 2 head-groups (4 heads each).
Each core computes, for its (batch, head-group):
    qkv projection (its heads' columns of Wqkv), per-head attention
    (softmax over full n=2048), and a partial output projection
    (its heads' rows of Wproj). Host sums the two partial outputs per
    batch (row-parallel linear unshard) and concatenates batches.

All matmul operands are float32r (full-rate PE, ~1e-4 relative rounding);
PSUM accumulation is fp32. Scores are computed transposed (ET[nk, nq]) so
no on-chip transposes are needed; softmax denominators come from an extra
ones-column appended to V (row 96 of the PV accumulator). exp() skips the
usual max-subtraction: logits/sqrt(e) for this problem are bounded (~|2|),
far from fp32 overflow.
"""

import os

import numpy as np

import concourse.bacc as bacc
import concourse.mybir as mybir
import concourse.tile as tile
from concourse.bass_utils import run_bass_kernel_spmd

B, N, E = 4, 2048, 768
H = 8          # total heads
HL = 4         # heads per core
D = E // H     # 96
DH = D + 1     # 97 (with denominator column)
KB = E // 128  # 6 contraction blocks
NB = N // 128  # 16 row blocks
NC = 8         # cores
SCALE = float(E) ** -0.5

F32 = mybir.dt.float32
F32R = mybir.dt.float32r
AF = mybir.ActivationFunctionType

_COMPILED = None
LAST_EXEC_NS = None
LAST_RESULTS = None


def _build():
    nc = bacc.Bacc("TRN2", target_bir_lowering=False, debug=False)

    xT_d = nc.dram_tensor("xT", [E, N], F32, kind="ExternalInput")
    wq_d = nc.dram_tensor("wq", [E, HL * D], F32, kind="ExternalInput")
    wk_d = nc.dram_tensor("wk", [E, HL * D], F32, kind="ExternalInput")
    wv_d = nc.dram_tensor("wv", [E, HL * DH], F32, kind="ExternalInput")
    bq_d = nc.dram_tensor("bq", [D, HL], F32, kind="ExternalInput")
    bk_d = nc.dram_tensor("bk", [D, HL], F32, kind="ExternalInput")
    bv_d = nc.dram_tensor("bv", [1, HL * DH], F32, kind="ExternalInput")
    wp_d = nc.dram_tensor("wp", [HL * D, E], F32, kind="ExternalInput")
    bp_d = nc.dram_tensor("bp", [1, E], F32, kind="ExternalInput")
    ones_d = nc.dram_tensor("ones", [1, 128], F32, kind="ExternalInput")
    out_d = nc.dram_tensor("out", [N, E], F32, kind="ExternalOutput")

    ot_d = nc.dram_tensor("ot_stage", [HL, D, N], F32R)

    with tile.TileContext(nc) as tc:
        with (
            tc.tile_pool(name="const", bufs=1) as cpool,
            tc.tile_pool(name="xt", bufs=1) as xpool,
            tc.tile_pool(name="qk", bufs=2) as qkpool,
            tc.tile_pool(name="vh", bufs=1) as vpool,
            tc.tile_pool(name="pt", bufs=3) as ptpool,
            tc.tile_pool(name="nrm", bufs=2) as npool,
            tc.tile_pool(name="outs", bufs=3) as opool,
            tc.tile_pool(name="pp", bufs=2, space="PSUM") as pp,
            tc.tile_pool(name="pattn", bufs=1, space="PSUM") as pattn,
        ):
            # ---- constants ----
            wq_sb = []
            wk_sb = []
            wv_sb = []
            for kb in range(KB):
                t = cpool.tile([128, HL * D], F32R, tag=f"wq{kb}")
                nc.gpsimd.dma_start(t[:], wq_d[kb * 128:(kb + 1) * 128, :])
                wq_sb.append(t)
                t = cpool.tile([128, HL * D], F32R, tag=f"wk{kb}")
                nc.gpsimd.dma_start(t[:], wk_d[kb * 128:(kb + 1) * 128, :])
                wk_sb.append(t)
                t = cpool.tile([128, HL * DH], F32R, tag=f"wv{kb}")
                nc.gpsimd.dma_start(t[:], wv_d[kb * 128:(kb + 1) * 128, :])
                wv_sb.append(t)
            wp_sb = []
            for h in range(HL):
                t = cpool.tile([D, E], F32R, tag=f"wp{h}")
                nc.gpsimd.dma_start(t[:], wp_d[h * D:(h + 1) * D, :])
                wp_sb.append(t)
            bq_sb = cpool.tile([D, HL], F32, tag="bq")
            nc.sync.dma_start(bq_sb[:], bq_d[:])
            bk_sb = cpool.tile([D, HL], F32, tag="bk")
            nc.sync.dma_start(bk_sb[:], bk_d[:])
            bv_sb = cpool.tile([1, HL * DH], F32R, tag="bv")
            nc.gpsimd.dma_start(bv_sb[:], bv_d[:])
            bp_sb = cpool.tile([1, E], F32R, tag="bp")
            nc.gpsimd.dma_start(bp_sb[:], bp_d[:])
            ones_sb = cpool.tile([1, 128], F32R, tag="ones")
            nc.gpsimd.dma_start(ones_sb[:], ones_d[:])

            xT_sb = []
            for kb in range(KB):
                t = xpool.tile([128, N], F32R, tag=f"xt{kb}")
                nc.gpsimd.dma_start(t[:], xT_d[kb * 128:(kb + 1) * 128, :])
                xT_sb.append(t)

            # ---- V-hat projection: vhat[nb] [128, HL*97] ----
            vhat = []
            with nc.named_scope("vproj"):
                for nb in range(NB):
                    ps = pp.tile([128, 512], F32, tag="pp")
                    for kb in range(KB):
                        nc.tensor.matmul(
                            ps[:, 0:HL * DH],
                            xT_sb[kb][:, nb * 128:(nb + 1) * 128],
                            wv_sb[kb][:],
                            start=(kb == 0),
                            stop=False,
                        )
                    nc.tensor.matmul(
                        ps[:, 0:HL * DH],
                        ones_sb[:],
                        bv_sb[:],
                        start=False,
                        stop=True,
                    )
                    vt = vpool.tile([128, HL * DH], F32R, tag=f"vh{nb}")
                    nc.vector.tensor_copy(vt[:], ps[:, 0:HL * DH])
                    vhat.append(vt)

            # ---- per-head: project qT/kT, attention, normalize ----
            for h in range(HL):
                with nc.named_scope(f"qkproj{h}"):
                    qT = qkpool.tile([D, N], F32R, tag="qT")
                    kT = qkpool.tile([D, N], F32R, tag="kT")
                    for cn, (w_sb, b_sb, dst, sc) in enumerate(
                        [(wq_sb, bq_sb, qT, SCALE), (wk_sb, bk_sb, kT, 1.0)]
                    ):
                        for c in range(4):
                            ps = pp.tile([128, 512], F32, tag="pp")
                            for kb in range(KB):
                                nc.tensor.matmul(
                                    ps[0:D, :],
                                    w_sb[kb][:, h * D:(h + 1) * D],
                                    xT_sb[kb][:, c * 512:(c + 1) * 512],
                                    start=(kb == 0),
                                    stop=(kb == KB - 1),
                                )
                            nc.scalar.activation(
                                dst[:, c * 512:(c + 1) * 512],
                                ps[0:D, :],
                                AF.Identity,
                                bias=b_sb[:, h:h + 1],
                                scale=sc,
                            )

                for qh in range(2):
                    with nc.named_scope(f"attn{h}_{qh}"):
                        acc = pattn.tile([DH, 1024], F32, tag="acc")
                        for kb in range(NB):
                            et = pattn.tile([128, 1024], F32, tag="et", bufs=2)
                            for j in range(2):
                                c = 2 * qh + j
                                nc.tensor.matmul(
                                    et[:, j * 512:(j + 1) * 512],
                                    kT[:, kb * 128:(kb + 1) * 128],
                                    qT[:, c * 512:(c + 1) * 512],
                                    start=True,
                                    stop=True,
                                )
                            pt = ptpool.tile([128, 1024], F32R, tag="pt")
                            nc.scalar.activation(pt[:], et[:], AF.Exp)
                            for j in range(2):
                                nc.tensor.matmul(
                                    acc[:, j * 512:(j + 1) * 512],
                                    vhat[kb][:, h * DH:(h + 1) * DH],
                                    pt[:, j * 512:(j + 1) * 512],
                                    start=(kb == 0),
                                    stop=(kb == NB - 1),
                                )

                    with nc.named_scope(f"norm{h}_{qh}"):
                        acc_sb = npool.tile([DH, 1024], F32, tag="acc_sb")
                        nc.vector.tensor_copy(acc_sb[:], acc[:])
                        for j in range(2):
                            c = 2 * qh + j
                            rec = npool.tile([1, 512], F32R, tag="rec")
                            with nc.allow_low_precision(
                                reason="f32r reciprocal, ~1e-4 rel is fine here"
                            ):
                                nc.vector.reciprocal(
                                    rec[:], acc_sb[D:DH, j * 512:(j + 1) * 512]
                                )
                            bc = pp.tile([128, 512], F32, tag="pp")
                            nc.tensor.matmul(
                                bc[0:D, :], ones_sb[:, 0:D], rec[:],
                                start=True, stop=True,
                            )
                            ot = npool.tile([D, 512], F32R, tag="ot")
                            nc.vector.tensor_tensor(
                                ot[:],
                                acc_sb[0:D, j * 512:(j + 1) * 512],
                                bc[0:D, :],
                                mybir.AluOpType.mult,
                            )
                            nc.sync.dma_start(
                                ot_d[h, :, c * 512:(c + 1) * 512], ot[:]
                            )

        # ---- output projection out[n, e] = sum_h OT_h^T @ Wp_h + bp ----
        with (
            tc.tile_pool(name="fin", bufs=3) as fpool,
            tc.tile_pool(name="pout", bufs=2, space="PSUM") as pout,
        ):
            for nb in range(NB):
                otn = fpool.tile([D, HL * 128], F32R, tag="otn")
                for h in range(HL):
                    nc.sync.dma_start(
                        otn[:, h * 128:(h + 1) * 128],
                        ot_d[h, :, nb * 128:(nb + 1) * 128],
                    )
                po = pout.tile([128, E], F32, tag="po")
                for off, w in [(0, 512), (512, 256)]:
                    for h in range(HL):
                        nc.tensor.matmul(
                            po[:, off:off + w],
                            otn[:, h * 128:(h + 1) * 128],
                            wp_sb[h][:, off:off + w],
                            start=(h == 0),
                            stop=False,
                        )
                    nc.tensor.matmul(
                        po[:, off:off + w],
                        ones_sb[:],
                        bp_sb[:, off:off + w],
                        start=False,
                        stop=True,
                    )
                osb = fpool.tile([128, E], F32, tag="osb")
                nc.vector.tensor_copy(osb[:], po[:])
                nc.sync.dma_start(out_d[nb * 128:(nb + 1) * 128, :], osb[:])

    nc.compile()
    return nc


def _shard(x, Wqkv, bqkv, Wproj, bproj):
    """Build per-core input maps. Core c -> (batch c//2, head-group c%2)."""
    Wr = np.ascontiguousarray(Wqkv.reshape(E, H, D, 3))
    br = np.ascontiguousarray(bqkv.reshape(H, D, 3))
    ones = np.ones((1, 128), np.float32)
    in_maps = []
    for c in range(NC):
        bb, hg = divmod(c, 2)
        hs = slice(hg * HL, (hg + 1) * HL)
        wq = np.ascontiguousarray(Wr[:, hs, :, 0].reshape(E, HL * D))
        wk = np.ascontiguousarray(Wr[:, hs, :, 1].reshape(E, HL * D))
        wv = np.zeros((E, HL, DH), np.float32)
        wv[:, :, :D] = Wr[:, hs, :, 2]
        bq = np.ascontiguousarray((br[hs, :, 0] * SCALE).T)  # [D, HL], pre-scaled
        bk = np.ascontiguousarray(br[hs, :, 1].T)
        bv = np.zeros((HL, DH), np.float32)
        bv[:, :D] = br[hs, :, 2]
        bv[:, D] = 1.0  # denominator ones column
        wp = np.ascontiguousarray(Wproj[hg * HL * D:(hg + 1) * HL * D, :])
        bp = bproj if hg == 0 else np.zeros_like(bproj)
        in_maps.append({
            "xT": np.ascontiguousarray(x[bb].T),
            "wq": wq,
            "wk": wk,
            "wv": np.ascontiguousarray(wv.reshape(E, HL * DH)),
            "bq": bq,
            "bk": bk,
            "bv": np.ascontiguousarray(bv.reshape(1, HL * DH)),
            "wp": wp,
            "bp": np.ascontiguousarray(bp.reshape(1, E)),
            "ones": ones,
        })
    return in_maps


def kernel(x, Wqkv, bqkv, Wproj, bproj):
    global _COMPILED, LAST_EXEC_NS, LAST_RESULTS
    x = np.asarray(x, dtype=np.float32)
    Wqkv = np.asarray(Wqkv, dtype=np.float32)
    bqkv = np.asarray(bqkv, dtype=np.float32)
    Wproj = np.asarray(Wproj, dtype=np.float32)
    bproj = np.asarray(bproj, dtype=np.float32)

    if _COMPILED is None:
        _COMPILED = _build()
    nc = _COMPILED

    in_maps = _shard(x, Wqkv, bqkv, Wproj, bproj)
    trace = bool(int(os.environ.get("BASS_MHA_TRACE", "0")))
    res = run_bass_kernel_spmd(nc, in_maps, list(range(NC)), trace=trace)
    LAST_EXEC_NS = res.exec_time_ns
    LAST_RESULTS = res

    out = np.empty((B, N, E), np.float32)
    for bb in range(B):
        out[bb] = res.results[2 * bb]["out"] + res.results[2 * bb + 1]["out"]
    return out


# revision 8
# speedup vs baseline: 1.4123x; 1.4123x over previous
"""Multi-head attention TRN2 kernel (b=4, n=2048, e=768, h=8 heads, d=96).

Sharding: 8 cores = 4 batches x 2 head-groups (4 heads each).
Each core computes, for its (batch, head-group):
    qkv projection (its heads' columns of Wqkv), per-head attention
    (softmax over full n=2048), and a partial output projection
    (its heads' rows of Wproj). Host sums the two partial outputs per
    batch (row-parallel linear unshard) and concatenates batches.

All matmul operands are float32r (full-rate PE, ~1e-4 relative rounding);
PSUM accumulation is fp32. Scores are computed transposed (ET[nk, nq]) so
no on-chip transposes are needed; softmax denominators come from an extra
ones-column appended to V (row 96 of the PV accumulator). exp() skips the
usual max-subtraction: logits/sqrt(e) for this problem are bounded (~|2|),
far from fp32 overflow.
"""

import os

import numpy as np

import concourse.bacc as bacc
import concourse.mybir as mybir
import concourse.tile as tile
from concourse.bass_utils import run_bass_kernel_spmd

B, N, E = 4, 2048, 768
H = 8          # total heads
HL = 4         # heads per core
D = E // H     # 96
DH = D + 1     # 97 (with denominator column)
KB = E // 128  # 6 contraction blocks
NB = N // 128  # 16 row blocks
NC = 8         # cores
SCALE = float(E) ** -0.5

F32 = mybir.dt.float32
F32R = mybir.dt.float32r
AF = mybir.ActivationFunctionType
MULT = mybir.AluOpType.mult
ADD = mybir.AluOpType.add

_COMPILED = None
LAST_EXEC_NS = None
LAST_RESULTS = None


def _device_reset():
    """Recover a wedged NeuronCore (NRT_EXEC_UNIT_UNRECOVERABLE) via axon."""
    try:
        import ctypes
        import time

        import jax

        jax.devices()
        lib = ctypes.CDLL("/opt/axon/libaxon_pjrt.so")
        lib.axon_reset.restype = ctypes.c_int64
        lib.axon_reset()
        time.sleep(3)
    except Exception:
        pass


def _build():
    nc = bacc.Bacc("TRN2", target_bir_lowering=False, debug=False)

    xT_d = nc.dram_tensor("xT", [E, N], F32, kind="ExternalInput")
    wq_d = nc.dram_tensor("wq", [E, HL * D], F32, kind="ExternalInput")
    wk_d = nc.dram_tensor("wk", [E, HL * D], F32, kind="ExternalInput")
    wv_d = nc.dram_tensor("wv", [E, HL * DH], F32, kind="ExternalInput")
    bq_d = nc.dram_tensor("bq", [D, HL], F32, kind="ExternalInput")
    bk_d = nc.dram_tensor("bk", [D, HL], F32, kind="ExternalInput")
    bv_d = nc.dram_tensor("bv", [1, HL * DH], F32, kind="ExternalInput")
    wp_d = nc.dram_tensor("wp", [HL * D, E], F32, kind="ExternalInput")
    bp_d = nc.dram_tensor("bp", [1, E], F32, kind="ExternalInput")
    ones_d = nc.dram_tensor("ones", [1, 128], F32, kind="ExternalInput")
    out_d = nc.dram_tensor("out", [N, E], F32, kind="ExternalOutput")

    ot_d = nc.dram_tensor("ot_stage", [HL, D, N], F32R)

    with tile.TileContext(nc) as tc:
        with (
            tc.tile_pool(name="const", bufs=1) as cpool,
            tc.tile_pool(name="xt", bufs=1) as xpool,
            tc.tile_pool(name="qk", bufs=2) as qkpool,
            tc.tile_pool(name="vh", bufs=1) as vpool,
            tc.tile_pool(name="pt", bufs=3) as ptpool,
            tc.tile_pool(name="nrm", bufs=2) as npool,
            tc.tile_pool(name="pp", bufs=2, space="PSUM") as pp,
            tc.tile_pool(name="pattn", bufs=1, space="PSUM") as pattn,
        ):
            # ---- constants ----
            wq_sb = []
            wk_sb = []
            wv_sb = []
            for kb in range(KB):
                t = cpool.tile([128, HL * D], F32R, tag=f"wq{kb}")
                nc.gpsimd.dma_start(t[:], wq_d[kb * 128:(kb + 1) * 128, :])
                wq_sb.append(t)
                t = cpool.tile([128, HL * D], F32R, tag=f"wk{kb}")
                nc.gpsimd.dma_start(t[:], wk_d[kb * 128:(kb + 1) * 128, :])
                wk_sb.append(t)
                t = cpool.tile([128, HL * DH], F32R, tag=f"wv{kb}")
                nc.gpsimd.dma_start(t[:], wv_d[kb * 128:(kb + 1) * 128, :])
                wv_sb.append(t)
            wp_sb = []
            for h in range(HL):
                t = cpool.tile([D, E], F32R, tag=f"wp{h}")
                nc.gpsimd.dma_start(t[:], wp_d[h * D:(h + 1) * D, :])
                wp_sb.append(t)
            bq_sb = cpool.tile([D, HL], F32, tag="bq")
            nc.sync.dma_start(bq_sb[:], bq_d[:])
            bk_sb = cpool.tile([D, HL], F32, tag="bk")
            nc.sync.dma_start(bk_sb[:], bk_d[:])
            bv_sb = cpool.tile([1, HL * DH], F32R, tag="bv")
            nc.gpsimd.dma_start(bv_sb[:], bv_d[:])
            bp_sb = cpool.tile([1, E], F32R, tag="bp")
            nc.gpsimd.dma_start(bp_sb[:], bp_d[:])
            ones_sb = cpool.tile([1, 128], F32R, tag="ones")
            nc.gpsimd.dma_start(ones_sb[:], ones_d[:])

            # xT loads, chunked by column so downstream matmuls start early
            xT_sb = []
            for kb in range(KB):
                t = xpool.tile([128, N], F32R, tag=f"xt{kb}")
                xT_sb.append(t)
            for c in range(4):
                for kb in range(KB):
                    nc.gpsimd.dma_start(
                        xT_sb[kb][:, c * 512:(c + 1) * 512],
                        xT_d[kb * 128:(kb + 1) * 128, c * 512:(c + 1) * 512],
                    )

            # broadcast bias tiles (one K=1 matmul each, reused everywhere)
            bvb_sb = cpool.tile([128, HL * DH], F32, tag="bvb")
            ps = pp.tile([128, 512], F32, tag="pp")
            nc.tensor.matmul(ps[:, 0:HL * DH], ones_sb[:], bv_sb[:], start=True, stop=True)
            nc.vector.tensor_copy(bvb_sb[:], ps[:, 0:HL * DH])
            bpb_sb = cpool.tile([128, E], F32, tag="bpb")
            for off, w in [(0, 512), (512, 256)]:
                ps = pp.tile([128, 512], F32, tag="pp")
                nc.tensor.matmul(
                    ps[:, 0:w], ones_sb[:], bp_sb[:, off:off + w], start=True, stop=True
                )
                nc.vector.tensor_copy(bpb_sb[:, off:off + w], ps[:, 0:w])

            # ---- V-hat projection: vhat[nb] [128, HL*97] (V + denom column) ----
            vhat = []
            with nc.named_scope("vproj"):
                for nb in range(NB):
                    ps = pp.tile([128, 512], F32, tag="pp")
                    for kb in range(KB):
                        nc.tensor.matmul(
                            ps[:, 0:HL * DH],
                            xT_sb[kb][:, nb * 128:(nb + 1) * 128],
                            wv_sb[kb][:],
                            start=(kb == 0),
                            stop=(kb == KB - 1),
                        )
                    vt = vpool.tile([128, HL * DH], F32R, tag=f"vh{nb}")
                    nc.vector.tensor_tensor(vt[:], ps[:, 0:HL * DH], bvb_sb[:], ADD)
                    vhat.append(vt)

            # ---- per-head: project qT/kT, attention; norm deferred one slot ----
            def emit_norm(job):
                h, qh, acc_sb = job
                with nc.named_scope(f"norm{h}_{qh}"):
                    sums = npool.tile([1, 1024], F32, tag="sums")
                    nc.vector.tensor_copy(sums[:], acc_sb[D:DH, :])
                    rec32 = npool.tile([1, 1024], F32, tag="rec32")
                    nc.vector.reciprocal_approx_fast(rec32[:], sums[:])
                    rec = npool.tile([1, 1024], F32R, tag="rec")
                    nc.vector.tensor_copy(rec[:], rec32[:])
                    for j in range(2):
                        c = 2 * qh + j
                        bc = pp.tile([128, 512], F32, tag="pp")
                        nc.tensor.matmul(
                            bc[0:D, :],
                            ones_sb[:, 0:D],
                            rec[:, j * 512:(j + 1) * 512],
                            start=True,
                            stop=True,
                        )
                        ot = npool.tile([D, 512], F32R, tag="ot")
                        nc.vector.tensor_tensor(
                            ot[:], acc_sb[0:D, j * 512:(j + 1) * 512], bc[0:D, :], MULT
                        )
                        nc.sync.dma_start(ot_d[h, :, c * 512:(c + 1) * 512], ot[:])

            pending = None
            for h in range(HL):
                with nc.named_scope(f"qkproj{h}"):
                    qT = qkpool.tile([D, N], F32R, tag="qT")
                    kT = qkpool.tile([D, N], F32R, tag="kT")
                    for w_sb, b_sb, dst, sc in [
                        (wq_sb, bq_sb, qT, SCALE),
                        (wk_sb, bk_sb, kT, 1.0),
                    ]:
                        for c in range(4):
                            ps = pp.tile([128, 512], F32, tag="pp")
                            for kb in range(KB):
                                nc.tensor.matmul(
                                    ps[0:D, :],
                                    w_sb[kb][:, h * D:(h + 1) * D],
                                    xT_sb[kb][:, c * 512:(c + 1) * 512],
                                    start=(kb == 0),
                                    stop=(kb == KB - 1),
                                )
                            nc.scalar.activation(
                                dst[:, c * 512:(c + 1) * 512],
                                ps[0:D, :],
                                AF.Identity,
                                bias=b_sb[:, h:h + 1],
                                scale=sc,
                            )

                for qh in range(2):
                    with nc.named_scope(f"attn{h}_{qh}"):
                        acc = pattn.tile([DH, 1024], F32, tag="acc")
                        for kb in range(NB):
                            et = pattn.tile([128, 1024], F32, tag="et", bufs=2)
                            for j in range(2):
                                c = 2 * qh + j
                                nc.tensor.matmul(
                                    et[:, j * 512:(j + 1) * 512],
                                    kT[:, kb * 128:(kb + 1) * 128],
                                    qT[:, c * 512:(c + 1) * 512],
                                    start=True,
                                    stop=True,
                                )
                            pt = ptpool.tile([128, 1024], F32R, tag="pt")
                            nc.scalar.activation(pt[:], et[:], AF.Exp)
                            for j in range(2):
                                nc.tensor.matmul(
                                    acc[:, j * 512:(j + 1) * 512],
                                    vhat[kb][:, h * DH:(h + 1) * DH],
                                    pt[:, j * 512:(j + 1) * 512],
                                    start=(kb == 0),
                                    stop=(kb == NB - 1),
                                )
                            if kb == 6 and pending is not None:
                                emit_norm(pending)
                                pending = None
                        acc_sb = npool.tile([DH, 1024], F32, tag="acc_sb")
                        nc.vector.tensor_copy(acc_sb[:], acc[:])
                        pending = (h, qh, acc_sb)
            emit_norm(pending)

        # ---- output projection out[n, e] = sum_h OT_h^T @ Wp_h + bp ----
        with (
            tc.tile_pool(name="fin", bufs=3) as fpool,
            tc.tile_pool(name="pout", bufs=2, space="PSUM") as pout,
        ):
            for nb in range(NB):
                otn = fpool.tile([D, HL * 128], F32R, tag="otn")
                for h in range(HL):
                    nc.sync.dma_start(
                        otn[:, h * 128:(h + 1) * 128],
                        ot_d[h, :, nb * 128:(nb + 1) * 128],
                    )
                po = pout.tile([128, E], F32, tag="po")
                for off, w in [(0, 512), (512, 256)]:
                    for h in range(HL):
                        nc.tensor.matmul(
                            po[:, off:off + w],
                            otn[:, h * 128:(h + 1) * 128],
                            wp_sb[h][:, off:off + w],
                            start=(h == 0),
                            stop=(h == HL - 1),
                        )
                osb = fpool.tile([128, E], F32, tag="osb")
                nc.vector.tensor_tensor(osb[:], po[:], bpb_sb[:], ADD)
                nc.sync.dma_start(out_d[nb * 128:(nb + 1) * 128, :], osb[:])

    nc.compile()
    return nc


def _shard(x, Wqkv, bqkv, Wproj, bproj):
    """Build per-core input maps. Core c -> (batch c//2, head-group c%2)."""
    Wr = np.ascontiguousarray(Wqkv.reshape(E, H, D, 3))
    br = np.ascontiguousarray(bqkv.reshape(H, D, 3))
    ones = np.ones((1, 128), np.float32)
    in_maps = []
    for c in range(NC):
        bb, hg = divmod(c, 2)
        hs = slice(hg * HL, (hg + 1) * HL)
        wq = np.ascontiguousarray(Wr[:, hs, :, 0].reshape(E, HL * D))
        wk = np.ascontiguousarray(Wr[:, hs, :, 1].reshape(E, HL * D))
        wv = np.zeros((E, HL, DH), np.float32)
        wv[:, :, :D] = Wr[:, hs, :, 2]
        bq = np.ascontiguousarray((br[hs, :, 0] * SCALE).T)  # [D, HL], pre-scaled
        bk = np.ascontiguousarray(br[hs, :, 1].T)
        bv = np.zeros((HL, DH), np.float32)
        bv[:, :D] = br[hs, :, 2]
        bv[:, D] = 1.0  # denominator ones column
        wp = np.ascontiguousarray(Wproj[hg * HL * D:(hg + 1) * HL * D, :])
        bp = bproj if hg == 0 else np.zeros_like(bproj)
        in_maps.append({
            "xT": np.ascontiguousarray(x[bb].T),
            "wq": wq,
            "wk": wk,
            "wv": np.ascontiguousarray(wv.reshape(E, HL * DH)),
            "bq": bq,
            "bk": bk,
            "bv": np.ascontiguousarray(bv.reshape(1, HL * DH)),
            "wp": wp,
            "bp": np.ascontiguousarray(bp.reshape(1, E)),
            "ones": ones,
        })
    return in_maps


def kernel(x, Wqkv, bqkv, Wproj, bproj):
    global _COMPILED, LAST_EXEC_NS, LAST_RESULTS
    x = np.asarray(x, dtype=np.float32)
    Wqkv = np.asarray(Wqkv, dtype=np.float32)
    bqkv = np.asarray(bqkv, dtype=np.float32)
    Wproj = np.asarray(Wproj, dtype=np.float32)
    bproj = np.asarray(bproj, dtype=np.float32)

    if _COMPILED is None:
        _COMPILED = _build()
    nc = _COMPILED

    in_maps = _shard(x, Wqkv, bqkv, Wproj, bproj)
    trace = bool(int(os.environ.get("BASS_MHA_TRACE", "0")))
    try:
        res = run_bass_kernel_spmd(nc, in_maps, list(range(NC)), trace=trace)
    except Exception:
        _device_reset()
        res = run_bass_kernel_spmd(nc, in_maps, list(range(NC)), trace=trace)
    LAST_EXEC_NS = res.exec_time_ns
    LAST_RESULTS = res

    out = np.empty((B, N, E), np.float32)
    for bb in range(B):
        out[bb] = res.results[2 * bb]["out"] + res.results[2 * bb + 1]["out"]
    return out
